# revision 20
# baseline (speedup 1.0000x reference)
"""AttnDecoderRNN single-step on 8 Trainium2 NeuronCores.

Strategy (tensor-parallel, per the vocab-sharding hint):
- Embedding table is vocab-sharded (Megatron style): each core gathers its
  shard's rows (missing rows hit a zero row) and an AllReduce(add) rebuilds
  the full embedded activations on every core.
- W_red / W_comb / GRU (W_ih, W_hh) are column-sharded: each core computes a
  128-feature slice of the output for all 64 batch rows; AllGather / AllToAll
  collectives re-assemble full activations where needed.
- Attention (scores, softmax, context) is batch-sharded: each core handles 8
  batch rows with its slice of encoder_outputs.
- W_out / log_softmax are vocab-sharded: each core computes logits for 6283
  vocab columns, local (max, sumexp) stats are AllGathered, and each core
  normalizes its slice on-device.

All activations are kept feature-major ("transposed", [features(p), batch(f)])
so the PE contraction dim is always on partitions.
"""

import sys
for _p in ("/opt/trn_rl_repo", "/root/.axon_site/_ro/trn_rl_repo"):
    if _p not in sys.path:
        sys.path.insert(0, _p)

import numpy as np
import ml_dtypes

import concourse.bass as bass
import concourse.mybir as mybir
from concourse import bacc, bass_utils, tile
from concourse.masks import make_identity

F32 = mybir.dt.float32
F32R = mybir.dt.float32r
BF16 = mybir.dt.bfloat16
I32 = mybir.dt.int32
AF = mybir.ActivationFunctionType
ALU = mybir.AluOpType

NC = 8           # cores
V = 50257        # vocab
E = H = 1024     # embed / hidden
B = 64           # batch
S = 512          # encoder positions
BL = B // NC     # local batch (8)
OSH = 6283       # vocab shard width (8*6283 = 50264 >= V)
OPAD = OSH * NC  # padded vocab
FSL = H // NC    # feature slice per core (128)
KC = 128         # contraction chunk
NEG = -1.0e30

PACK = True      # pack 4 batches into PE column groups for attention


def build_bass():
    nc = bacc.Bacc("TRN2", target_bir_lowering=False, debug=False, num_devices=NC)

    din = {}
    def inp(name, shape, dt=F32):
        din[name] = nc.dram_tensor(name, shape, dt, kind="ExternalInput").ap()
        return din[name]

    emb_sh = inp("emb_sh", [OSH + 1, E])
    idx_safe = inp("idx_safe", [B, 1], I32)
    hT_in = inp("hT_in", [128, 512])            # hiddenT chunk-packed
    encT_sc = inp("encT_sc", [BL, 8, 128, 512])  # [b, hc, h128, s512]
    enc_ct = inp("enc_ct", [BL, 4, 128, 1024])   # [b, sc, s128, h1024]
    lens_sp = inp("lens_sp", [128, 2])
    W_red_s = inp("W_red_s", [16, 128, 128])
    b_red_s = inp("b_red_s", [128, 1])
    W_comb_s = inp("W_comb_s", [16, 128, 128])
    b_comb_s = inp("b_comb_s", [128, 1])
    W_ih_s = inp("W_ih_s", [8, 128, 384])
    W_hh_s = inp("W_hh_s", [8, 128, 384])
    b_rz = inp("b_rz", [128, 2])
    b_ni = inp("b_ni", [128, 1])
    b_nh = inp("b_nh", [128, 1])
    W_out_s = inp("W_out_s", [8, 128, OSH], BF16)
    b_out_s = inp("b_out_s", [B, OSH])

    out_slice = nc.dram_tensor("out_slice", [B, OSH], F32, kind="ExternalOutput").ap()
    hidden_out = nc.dram_tensor("hidden_out", [B, H], F32, kind="ExternalOutput").ap()

    RG = [list(range(NC))]

    with tile.TileContext(nc) as tc:
        with (
            tc.tile_pool(name="dram", bufs=1, space="DRAM") as dram,
            tc.tile_pool(name="persist", bufs=1) as pp,
            tc.tile_pool(name="wtiles", bufs=4) as wp,
            tc.tile_pool(name="enc_sc", bufs=20) as ep_sc,
            tc.tile_pool(name="enc_ctp", bufs=4) as ep_ct,
            tc.tile_pool(name="wout", bufs=20) as op_,
            tc.tile_pool(name="ps_t", bufs=1, space="PSUM") as ps,
            tc.tile_pool(name="ps_big", bufs=3, space="PSUM") as psb,
            tc.tile_pool(name="ps_g", bufs=4, space="PSUM") as psg,
        ):
            # ---- constants -------------------------------------------------
            ident = pp.tile([128, 128], F32)
            make_identity(nc, ident[:])

            # ---- collective bounce buffers --------------------------------
            ar_emb_in = dram.tile([128, 512], F32)
            ar_emb_out = dram.tile([128, 512], F32)
            a2a_q_in = dram.tile([B, 128], F32)
            a2a_q_out = dram.tile([B, 128], F32)
            ag_ctx_in = dram.tile([BL, H], F32)
            ag_ctx_out = dram.tile([B, H], F32)
            ag_nh_in = dram.tile([FSL, B], F32)
            ag_nh_out = dram.tile([H, B], F32)
            ag_hn_in = dram.tile([FSL, B], F32)
            ag_hn_out = dram.tile([H, B], F32)
            ag_st_in = dram.tile([B, 2], F32)
            ag_st_out = dram.tile([B * NC, 2], F32)

            # ---- phase 0: embedding gather + AllReduce --------------------
            idx_sb = pp.tile([B, 1], I32)
            nc.scalar.dma_start(idx_sb[:], idx_safe[:])
            emb_nat = pp.tile([B, E], F32)
            nc.gpsimd.indirect_dma_start(
                out=emb_nat[:],
                out_offset=None,
                in_=emb_sh[:],
                in_offset=bass.IndirectOffsetOnAxis(ap=idx_sb[:, :1], axis=0),
            )
            # transpose the local partial embedding BEFORE the AllReduce so
            # no PE work sits after the collective on the critical path
            embT_loc = pp.tile([128, 512], F32)
            for c in range(8):
                pt = ps.tile([128, 128], F32, tag="ps_t", name="pt")[:, :B]
                nc.tensor.transpose(pt[:], emb_nat[:, 128 * c:128 * (c + 1)],
                                    ident[:B, :B])
                nc.vector.tensor_copy(embT_loc[:, B * c:B * (c + 1)], pt[:])
            nc.scalar.dma_start(ar_emb_in[:], embT_loc[:])
            nc.gpsimd.collective_compute(
                "AllReduce", ALU.add, replica_groups=RG,
                ins=[ar_emb_in.opt()], outs=[ar_emb_out.opt()],
            )
            embT_all = pp.tile([128, 512], F32)
            nc.scalar.dma_start(embT_all[:], ar_emb_out[:])

            hT_all = pp.tile([128, 512], F32)
            nc.scalar.dma_start(hT_all[:], hT_in[:])

            # ---- phase 1: W_red (column-sharded) --------------------------
            pq = psg.tile([128, B], F32, tag="ps_g", name="pq")
            for k in list(range(8, 16)) + list(range(8)):
                wt = wp.tile([128, 128], F32, tag="wred")
                nc.sync.dma_start(wt[:], W_red_s[k])
                rhs = embT_all if k < 8 else hT_all
                kc = k % 8
                nc.tensor.matmul(pq[:], lhsT=wt[:], rhs=rhs[:, B * kc:B * (kc + 1)],
                                 start=(k == 8), stop=(k == 7))
            b_red_sb = pp.tile([128, 1], F32)
            nc.scalar.dma_start(b_red_sb[:], b_red_s[:])
            q_sl = pp.tile([128, B], F32)
            nc.scalar.activation(q_sl[:], pq[:], AF.Identity, bias=b_red_sb[:, 0:1])

            # ---- phase 2: AllToAll q (feature-shard -> batch-shard) -------
            ptq = ps.tile([128, 128], F32, tag="ps_t", name="pt")[:B, :]
            nc.tensor.transpose(ptq[:], q_sl[:], ident[:])
            q_nat_sl = pp.tile([B, 128], F32)
            nc.vector.tensor_copy(q_nat_sl[:], ptq[:])
            nc.scalar.dma_start(a2a_q_in[:], q_nat_sl[:])
            nc.gpsimd.collective_compute(
                "AllToAll", ALU.bypass, replica_groups=RG,
                ins=[a2a_q_in.opt()], outs=[a2a_q_out.opt()],
            )
            q_own_nat = pp.tile([BL, E], F32)
            for c in range(NC):
                nc.scalar.dma_start(q_own_nat[:, 128 * c:128 * (c + 1)],
                                  a2a_q_out[BL * c:BL * (c + 1), :])
            # qT_all: [128, 8*8], h-chunk c at cols 8c
            qT_all = pp.tile([128, 64], F32)
            for c in range(8):
                pt = ps.tile([128, 128], F32, tag="ps_t", name="pt")[:, :BL]
                nc.tensor.transpose(pt[:], q_own_nat[:, 128 * c:128 * (c + 1)],
                                    ident[:BL, :BL])
                nc.vector.tensor_copy(qT_all[:, BL * c:BL * (c + 1)], pt[:])

            # ---- phase 3: attention (batch-sharded) -----------------------
            # Work on "spread" layouts: batch b=4r+j lives on partition 32j
            # (PE column groups); other partitions hold garbage and are
            # masked to harmless values or never read.
            lens_sp_sb = pp.tile([128, 2], F32)
            nc.scalar.dma_start(lens_sp_sb[:], lens_sp[:])
            iota_i = pp.tile([128, S], I32)
            nc.gpsimd.iota(iota_i[:], pattern=[[1, S]], base=0, channel_multiplier=0)
            iota_f = pp.tile([128, S], F32)
            nc.vector.tensor_copy(iota_f[:], iota_i[:])

            attnT_sp = []
            for r in range(2):
                pen = pp.tile([128, S], F32, tag=f"pen{r}", name=f"pen{r}")
                nc.vector.tensor_scalar(pen[:], iota_f[:], lens_sp_sb[:, r:r + 1],
                                        1e30, ALU.is_ge, ALU.mult)
                pss = psb.tile([128, S], F32, tag="ps_big", name="psbt")
                for c in range(8):
                    for j in range(4):
                        b = 4 * r + j
                        et = ep_sc.tile([128, S], F32, tag="encT")
                        nc.sync.dma_start(et[:], encT_sc[b, c])
                        nc.tensor.matmul(
                            pss[32 * j:32 * j + 1, :],
                            lhsT=qT_all[:, 8 * c + b:8 * c + b + 1],
                            rhs=et[:],
                            start=(c == 0), stop=(c == 7),
                            tile_position=(0, 32 * j),
                        )
                # masked softmax on the spread rows (free-dim ops, rowwise)
                sc_m = pp.tile([128, S], F32, tag=f"sc_m{r}", name=f"sc_m{r}")
                nc.vector.tensor_tensor(out=sc_m[:], in0=pss[:], in1=pen[:],
                                        op=ALU.subtract)
                mx = pp.tile([128, 1], F32, tag=f"mx{r}", name=f"mx{r}")
                nc.vector.reduce_max(mx[:], sc_m[:], axis=mybir.AxisListType.X)
                nmx = pp.tile([128, 1], F32, tag=f"nmx{r}", name=f"nmx{r}")
                nc.vector.tensor_scalar_mul(nmx[:], mx[:], -1.0)
                ssum = pp.tile([128, 1], F32, tag=f"ssum{r}", name=f"ssum{r}")
                nc.scalar.activation(sc_m[:], sc_m[:], AF.Exp, bias=nmx[:, 0:1],
                                     accum_out=ssum[:])
                sinv = pp.tile([128, 1], F32, tag=f"sinv{r}", name=f"sinv{r}")
                nc.vector.reciprocal(sinv[:], ssum[:])
                nc.vector.tensor_scalar_mul(sc_m[:], sc_m[:], sinv[:, 0:1])
                # transpose: attnT_sp[r][:, 128*sc + 32*j] = attn weights
                at = pp.tile([128, 512], F32, tag=f"attnT{r}", name=f"attnT{r}")
                for sc in range(4):
                    pt = ps.tile([128, 128], F32, tag="ps_t", name="pt")
                    nc.tensor.transpose(pt[:], sc_m[:, 128 * sc:128 * (sc + 1)],
                                        ident[:])
                    nc.vector.tensor_copy(at[:, 128 * sc:128 * (sc + 1)], pt[:])
                attnT_sp.append(at)

            for r in range(2):
                psc0 = psb.tile([128, 512], F32, tag="ps_big", name="psc0")
                psc1 = psb.tile([128, 512], F32, tag="ps_big", name="psc1")
                psc = [psc0, psc1]
                for sc in range(4):
                    for j in range(4):
                        b = 4 * r + j
                        ct = ep_ct.tile([128, 1024], F32, tag="enc_ct")
                        nc.sync.dma_start(ct[:], enc_ct[b, sc])
                        for ht in range(2):
                            nc.tensor.matmul(
                                psc[ht][32 * j:32 * j + 1, :],
                                lhsT=attnT_sp[r][:, 128 * sc + 32 * j:
                                                 128 * sc + 32 * j + 1],
                                rhs=ct[:, 512 * ht:512 * (ht + 1)],
                                start=(sc == 0), stop=(sc == 3),
                                tile_position=(0, 32 * j),
                            )
                for j in range(4):
                    stg = wp.tile([1, H], F32, tag="ctx_stage", name="stg", bufs=2)
                    for ht in range(2):
                        nc.vector.tensor_copy(stg[:1, 512 * ht:512 * (ht + 1)],
                                              psc[ht][32 * j:32 * j + 1, :])
                    nc.scalar.dma_start(ag_ctx_in[4 * r + j:4 * r + j + 1, :],
                                      stg[:1, :])

            # ---- AllGather context -> full, transpose ---------------------
            nc.gpsimd.collective_compute(
                "AllGather", ALU.bypass, replica_groups=RG,
                ins=[ag_ctx_in.opt()], outs=[ag_ctx_out.opt()],
            )
            ctx_full = pp.tile([B, H], F32)
            nc.scalar.dma_start(ctx_full[:], ag_ctx_out[:])
            ctxT_all = pp.tile([128, 512], F32)
            for c in range(8):
                pt = ps.tile([128, 128], F32, tag="ps_t", name="pt")[:, :B]
                nc.tensor.transpose(pt[:], ctx_full[:, 128 * c:128 * (c + 1)],
                                    ident[:B, :B])
                nc.vector.tensor_copy(ctxT_all[:, B * c:B * (c + 1)], pt[:])

            # ---- phase 4: W_comb (column-sharded) -------------------------
            pnh = psg.tile([128, B], F32, tag="ps_g", name="pnh")
            for k in list(range(8, 16)) + list(range(8)):
                wt = wp.tile([128, 128], F32, tag="wcomb")
                nc.sync.dma_start(wt[:], W_comb_s[k])
                rhs = ctxT_all if k < 8 else hT_all
                kc = k % 8
                nc.tensor.matmul(pnh[:], lhsT=wt[:], rhs=rhs[:, B * kc:B * (kc + 1)],
                                 start=(k == 8), stop=(k == 7))
            b_comb_sb = pp.tile([128, 1], F32)
            nc.scalar.dma_start(b_comb_sb[:], b_comb_s[:])
            nhT_own = pp.tile([128, B], F32)
            nc.scalar.activation(nhT_own[:], pnh[:], AF.Identity,
                                 bias=b_comb_sb[:, 0:1])

            # ---- phase 5: AllGather new_hidden ----------------------------
            nc.scalar.dma_start(ag_nh_in[:], nhT_own[:])
            nc.gpsimd.collective_compute(
                "AllGather", ALU.bypass, replica_groups=RG,
                ins=[ag_nh_in.opt()], outs=[ag_nh_out.opt()],
            )
            nhT_all = pp.tile([128, 512], F32)
            for c in range(8):
                nc.scalar.dma_start(nhT_all[:, B * c:B * (c + 1)],
                                  ag_nh_out[128 * c:128 * (c + 1), :])

            # ---- phase 6: GRU (column-sharded) ----------------------------
            b_rz_sb = pp.tile([128, 2], F32)
            nc.scalar.dma_start(b_rz_sb[:], b_rz[:])
            b_ni_sb = pp.tile([128, 1], F32)
            nc.scalar.dma_start(b_ni_sb[:], b_ni[:])
            b_nh_sb = pp.tile([128, 1], F32)
            nc.scalar.dma_start(b_nh_sb[:], b_nh[:])

            pr = psg.tile([128, B], F32, tag="ps_g", name="pr")
            pz = psg.tile([128, B], F32, tag="ps_g", name="pz")
            pni = psg.tile([128, B], F32, tag="ps_g", name="pni")
            phn = psg.tile([128, B], F32, tag="ps_g", name="phn")
            for k in range(8):
                wt = wp.tile([128, 384], F32, tag="wih", bufs=3, name="wtih")
                nc.sync.dma_start(wt[:], W_ih_s[k])
                rhsk = embT_all[:, B * k:B * (k + 1)]
                nc.tensor.matmul(pr[:], lhsT=wt[:, 0:128], rhs=rhsk,
                                 start=(k == 0), stop=False)
                nc.tensor.matmul(pz[:], lhsT=wt[:, 128:256], rhs=rhsk,
                                 start=(k == 0), stop=False)
                nc.tensor.matmul(pni[:], lhsT=wt[:, 256:384], rhs=rhsk,
                                 start=(k == 0), stop=(k == 7))
            for k in range(8):
                wt2 = wp.tile([128, 384], F32, tag="whh", bufs=3, name="wthh")
                nc.sync.dma_start(wt2[:], W_hh_s[k])
                rhsk = nhT_all[:, B * k:B * (k + 1)]
                nc.tensor.matmul(pr[:], lhsT=wt2[:, 0:128], rhs=rhsk,
                                 start=False, stop=(k == 7))
                nc.tensor.matmul(pz[:], lhsT=wt2[:, 128:256], rhs=rhsk,
                                 start=False, stop=(k == 7))
                nc.tensor.matmul(phn[:], lhsT=wt2[:, 256:384], rhs=rhsk,
                                 start=(k == 0), stop=(k == 7))
            r_sb = pp.tile([128, B], F32)
            nc.scalar.activation(r_sb[:], pr[:], AF.Sigmoid, bias=b_rz_sb[:, 0:1])
            z_sb = pp.tile([128, B], F32)
            nc.scalar.activation(z_sb[:], pz[:], AF.Sigmoid, bias=b_rz_sb[:, 1:2])
            ni_sb = pp.tile([128, B], F32)
            nc.scalar.activation(ni_sb[:], pni[:], AF.Identity, bias=b_ni_sb[:, 0:1])
            hn_sb = pp.tile([128, B], F32)
            nc.scalar.activation(hn_sb[:], phn[:], AF.Identity, bias=b_nh_sb[:, 0:1])

            rhn = pp.tile([128, B], F32)
            nc.vector.tensor_tensor(out=rhn[:], in0=r_sb[:], in1=hn_sb[:], op=ALU.mult)
            npre = pp.tile([128, B], F32)
            nc.vector.tensor_tensor(out=npre[:], in0=ni_sb[:], in1=rhn[:], op=ALU.add)
            n_sb = pp.tile([128, B], F32)
            nc.scalar.activation(n_sb[:], npre[:], AF.Tanh)
            # h_new = n + z*(nh - n)
            dlt = pp.tile([128, B], F32)
            nc.vector.tensor_tensor(out=dlt[:], in0=nhT_own[:], in1=n_sb[:],
                                    op=ALU.subtract)
            zd = pp.tile([128, B], F32)
            nc.vector.tensor_tensor(out=zd[:], in0=z_sb[:], in1=dlt[:], op=ALU.mult)
            hnT_own = pp.tile([128, B], F32)
            nc.vector.tensor_tensor(out=hnT_own[:], in0=n_sb[:], in1=zd[:], op=ALU.add)

            # ---- phase 7: AllGather h_new ---------------------------------
            nc.scalar.dma_start(ag_hn_in[:], hnT_own[:])
            nc.gpsimd.collective_compute(
                "AllGather", ALU.bypass, replica_groups=RG,
                ins=[ag_hn_in.opt()], outs=[ag_hn_out.opt()],
            )
            hnT_all = pp.tile([128, 512], F32)
            hnT_bf = pp.tile([128, 512], BF16)
            for c in range(8):
                nc.scalar.dma_start(hnT_all[:, B * c:B * (c + 1)],
                                  ag_hn_out[128 * c:128 * (c + 1), :])
                nc.vector.tensor_copy(hnT_bf[:, B * c:B * (c + 1)],
                                      hnT_all[:, B * c:B * (c + 1)])

            # hidden_out: transpose back to [B, H] and write
            hout_sb = pp.tile([B, H], F32)
            for c in range(8):
                pt = ps.tile([128, 128], F32, tag="ps_t", name="pt")[:B, :]
                nc.tensor.transpose(pt[:], hnT_all[:, B * c:B * (c + 1)], ident[:])
                nc.vector.tensor_copy(hout_sb[:, 128 * c:128 * (c + 1)], pt[:])
            nc.scalar.dma_start(hidden_out[:], hout_sb[:])

            # ---- phase 8: logits (vocab-sharded) --------------------------
            logits = pp.tile([B, OSH], F32)
            esums = pp.tile([B, 16], F32)
            n_tiles = (OSH + 511) // 512
            for nt in range(n_tiles):
                n0 = 512 * nt
                nsz = min(512, OSH - n0)
                pl = psb.tile([128, 512], F32, tag="ps_big", name="psbt")[:B, :]
                for k in range(8):
                    wt = op_.tile([128, 512], BF16, tag="wout")
                    nc.sync.dma_start(wt[:, :nsz], W_out_s[k, :, n0:n0 + nsz])
                    nc.tensor.matmul(pl[:, :nsz],
                                     lhsT=hnT_bf[:, B * k:B * (k + 1)],
                                     rhs=wt[:, :nsz], start=(k == 0),
                                     stop=(k == 7))
                bt = wp.tile([B, 512], F32, tag="bout", bufs=2)
                nc.sync.dma_start(bt[:, :nsz], b_out_s[:, n0:n0 + nsz])
                nc.vector.tensor_tensor(
                    out=logits[:, n0:n0 + nsz], in0=pl[:, :nsz],
                    in1=bt[:, :nsz], op=ALU.add)
                # per-tile exp + row-sum (no max shift: |logits| is O(10))
                pt = wp.tile([B, 512], F32, tag="probs", bufs=2)
                nc.scalar.activation(pt[:, :nsz], logits[:, n0:n0 + nsz], AF.Exp,
                                     accum_out=esums[:, nt:nt + 1])

            # ---- phase 9: log_softmax with global sum ---------------------
            sl = pp.tile([B, 1], F32)
            nc.vector.reduce_sum(sl[:], esums[:, :n_tiles],
                                 axis=mybir.AxisListType.X)
            nc.scalar.dma_start(ag_st_in[:, 0:1], sl[:])
            nc.gpsimd.collective_compute(
                "AllGather", ALU.bypass, replica_groups=RG,
                ins=[ag_st_in.opt()], outs=[ag_st_out.opt()],
            )
            stats_all = pp.tile([B, NC], F32)
            for rr in range(NC):
                nc.scalar.dma_start(stats_all[:, rr:rr + 1],
                                  ag_st_out[B * rr:B * (rr + 1), 0:1])
            sg = pp.tile([B, 1], F32)
            nc.vector.reduce_sum(sg[:], stats_all[:], axis=mybir.AxisListType.X)
            lse = pp.tile([B, 1], F32)
            nc.scalar.activation(lse[:], sg[:], AF.Ln)
            for nt in range(n_tiles):
                n0 = 512 * nt
                nsz = min(512, OSH - n0)
                nc.vector.tensor_scalar(logits[:, n0:n0 + nsz],
                                        logits[:, n0:n0 + nsz],
                                        lse[:, 0:1], None, ALU.subtract)
                nc.scalar.dma_start(out_slice[:, n0:n0 + nsz],
                                    logits[:, n0:n0 + nsz])

    nc.compile()
    return nc


_NC_CACHE = None


def _get_bass():
    global _NC_CACHE
    if _NC_CACHE is None:
        _NC_CACHE = build_bass()
    return _NC_CACHE


def _prep_in_maps(inputs, hidden, encoder_outputs, encoder_lengths, emb,
                  W_red, b_red, W_comb, b_comb, W_ih, b_ih, W_hh, b_hh,
                  W_out, b_out):
    f32 = np.float32
    idx = np.asarray(inputs).reshape(B).astype(np.int64)
    hT_in = np.ascontiguousarray(
        np.asarray(hidden, f32)[0].T.reshape(8, 128, B).transpose(1, 0, 2)
        .reshape(128, 512))
    enc = np.asarray(encoder_outputs, f32)
    W_out = np.asarray(W_out, f32)
    W_out_pad = np.zeros((H, OPAD), f32)
    W_out_pad[:, :V] = W_out
    b_out_pad = np.full((OPAD,), NEG, f32)
    b_out_pad[:V] = np.asarray(b_out, f32)
    W_red = np.asarray(W_red, f32)
    W_comb = np.asarray(W_comb, f32)
    W_ih = np.asarray(W_ih, f32)
    W_hh = np.asarray(W_hh, f32)
    b_ih = np.asarray(b_ih, f32)
    b_hh = np.asarray(b_hh, f32)
    emb = np.asarray(emb, f32)
    lens_all = np.asarray(encoder_lengths).reshape(B).astype(f32)

    in_maps = []
    for i in range(NC):
        lo = OSH * i
        hi = min(V, OSH * (i + 1))
        emb_sh = np.zeros((OSH + 1, E), f32)
        emb_sh[:hi - lo] = emb[lo:hi]
        loc = idx - lo
        idx_safe = np.where((loc >= 0) & (loc < hi - lo), loc, OSH) \
            .astype(np.int32).reshape(B, 1)

        bsl = slice(BL * i, BL * (i + 1))
        lens_sp = np.zeros((128, 2), f32)
        for r in range(2):
            for j in range(4):
                lens_sp[32 * j, r] = lens_all[BL * i + 4 * r + j]
        e = enc[:, bsl, :]                                   # [512, 8, 1024]
        encT_sc = np.ascontiguousarray(e.transpose(1, 2, 0)).reshape(BL, 8, 128, S)
        enc_ct = np.ascontiguousarray(e.transpose(1, 0, 2)).reshape(BL, 4, 128, H)

        fsl = slice(FSL * i, FSL * (i + 1))
        W_red_s = np.ascontiguousarray(W_red[:, fsl]).reshape(16, 128, 128)
        W_comb_s = np.ascontiguousarray(W_comb[:, fsl]).reshape(16, 128, 128)
        o0 = FSL * i
        cols = np.r_[o0:o0 + FSL, H + o0:H + o0 + FSL, 2 * H + o0:2 * H + o0 + FSL]
        W_ih_s = np.ascontiguousarray(W_ih[:, cols]).reshape(8, 128, 384)
        W_hh_s = np.ascontiguousarray(W_hh[:, cols]).reshape(8, 128, 384)
        b_rz = np.stack([b_ih[o0:o0 + FSL] + b_hh[o0:o0 + FSL],
                         b_ih[H + o0:H + o0 + FSL] + b_hh[H + o0:H + o0 + FSL]],
                        axis=1).astype(f32)
        b_ni = b_ih[2 * H + o0:2 * H + o0 + FSL].reshape(FSL, 1).astype(f32)
        b_nh = b_hh[2 * H + o0:2 * H + o0 + FSL].reshape(FSL, 1).astype(f32)
        W_out_s = W_out_pad[:, lo:lo + OSH].astype(ml_dtypes.bfloat16) \
            .reshape(8, 128, OSH)
        b_out_s = np.broadcast_to(b_out_pad[lo:lo + OSH], (B, OSH)).copy()

        in_maps.append({
            "emb_sh": emb_sh,
            "idx_safe": idx_safe,
            "hT_in": hT_in,
            "encT_sc": encT_sc,
            "enc_ct": enc_ct,
            "lens_sp": lens_sp,
            "W_red_s": W_red_s,
            "b_red_s": np.asarray(b_red, f32)[fsl].reshape(FSL, 1),
            "W_comb_s": W_comb_s,
            "b_comb_s": np.asarray(b_comb, f32)[fsl].reshape(FSL, 1),
            "W_ih_s": W_ih_s,
            "W_hh_s": W_hh_s,
            "b_rz": b_rz,
            "b_ni": b_ni,
            "b_nh": b_nh,
            "W_out_s": W_out_s,
            "b_out_s": b_out_s,
        })
    return in_maps


last_exec_time_ns = None


def kernel(_profile=False, **inputs):
    global last_exec_time_ns
    nc = _get_bass()
    in_maps = _prep_in_maps(**inputs)
    kw = {}
    if _profile:
        kw["trace"] = True
    res = bass_utils.run_bass_kernel_spmd(nc, in_maps, core_ids=list(range(NC)), **kw)
    last_exec_time_ns = res.exec_time_ns
    out = np.concatenate([r["out_slice"] for r in res.results], axis=1)[:, :V]
    hidden_out = res.results[0]["hidden_out"].reshape(1, B, H)
    return np.ascontiguousarray(out), hidden_out


# revision 21
# speedup vs baseline: 1.0757x; 1.0757x over previous
"""AttnDecoderRNN single-step on 8 Trainium2 NeuronCores.

Strategy (tensor-parallel, per the vocab-sharding hint):
- Embedding table is vocab-sharded (Megatron style): each core gathers its
  shard's rows (missing rows hit a zero row) and an AllReduce(add) rebuilds
  the full embedded activations on every core.
- W_red / W_comb / GRU (W_ih, W_hh) are column-sharded: each core computes a
  128-feature slice of the output for all 64 batch rows; AllGather / AllToAll
  collectives re-assemble full activations where needed.
- Attention (scores, softmax, context) is batch-sharded: each core handles 8
  batch rows with its slice of encoder_outputs.
- W_out / log_softmax are vocab-sharded: each core computes logits for 6283
  vocab columns, local (max, sumexp) stats are AllGathered, and each core
  normalizes its slice on-device.

All activations are kept feature-major ("transposed", [features(p), batch(f)])
so the PE contraction dim is always on partitions.
"""

import sys
for _p in ("/opt/trn_rl_repo", "/root/.axon_site/_ro/trn_rl_repo"):
    if _p not in sys.path:
        sys.path.insert(0, _p)

import numpy as np
import ml_dtypes

import concourse.bass as bass
import concourse.mybir as mybir
from concourse import bacc, bass_utils, tile
from concourse.masks import make_identity

F32 = mybir.dt.float32
F32R = mybir.dt.float32r
BF16 = mybir.dt.bfloat16
I32 = mybir.dt.int32
AF = mybir.ActivationFunctionType
ALU = mybir.AluOpType

NC = 8           # cores
V = 50257        # vocab
E = H = 1024     # embed / hidden
B = 64           # batch
S = 512          # encoder positions
BL = B // NC     # local batch (8)
OSH = 6283       # vocab shard width (8*6283 = 50264 >= V)
OPAD = OSH * NC  # padded vocab
FSL = H // NC    # feature slice per core (128)
KC = 128         # contraction chunk
NEG = -1.0e30

PACK = True      # pack 4 batches into PE column groups for attention


def build_bass():
    nc = bacc.Bacc("TRN2", target_bir_lowering=False, debug=False, num_devices=NC)

    din = {}
    def inp(name, shape, dt=F32):
        din[name] = nc.dram_tensor(name, shape, dt, kind="ExternalInput").ap()
        return din[name]

    emb_sh = inp("emb_sh", [OSH + 1, E])
    idx_safe = inp("idx_safe", [B, 1], I32)
    hT_in = inp("hT_in", [128, 512])            # hiddenT chunk-packed
    encT_sc = inp("encT_sc", [BL, 8, 128, 512])  # [b, hc, h128, s512]
    enc_ct = inp("enc_ct", [BL, 4, 128, 1024])   # [b, sc, s128, h1024]
    lens_sp = inp("lens_sp", [128, 2])
    W_red_s = inp("W_red_s", [16, 128, 128])
    b_red_s = inp("b_red_s", [128, 1])
    W_comb_s = inp("W_comb_s", [16, 128, 128])
    b_comb_s = inp("b_comb_s", [128, 1])
    W_ih_s = inp("W_ih_s", [8, 128, 384])
    W_hh_s = inp("W_hh_s", [8, 128, 384])
    b_rz = inp("b_rz", [128, 2])
    b_ni = inp("b_ni", [128, 1])
    b_nh = inp("b_nh", [128, 1])
    W_out_s = inp("W_out_s", [8, 128, OSH], BF16)
    b_out_s = inp("b_out_s", [B, OSH])

    out_slice = nc.dram_tensor("out_slice", [B, OSH], F32, kind="ExternalOutput").ap()
    hidden_out = nc.dram_tensor("hidden_out", [B, H], F32, kind="ExternalOutput").ap()

    RG = [list(range(NC))]

    with tile.TileContext(nc) as tc:
        with (
            tc.tile_pool(name="dram", bufs=1, space="DRAM") as dram,
            tc.tile_pool(name="persist", bufs=1) as pp,
            tc.tile_pool(name="wtiles", bufs=4) as wp,
            tc.tile_pool(name="enc_sc", bufs=20) as ep_sc,
            tc.tile_pool(name="enc_ctp", bufs=4) as ep_ct,
            tc.tile_pool(name="wout", bufs=12) as op_,
            tc.tile_pool(name="ps_t", bufs=1, space="PSUM") as ps,
            tc.tile_pool(name="ps_big", bufs=3, space="PSUM") as psb,
            tc.tile_pool(name="ps_g", bufs=4, space="PSUM") as psg,
        ):
            # ---- constants -------------------------------------------------
            ident = pp.tile([128, 128], F32)
            make_identity(nc, ident[:])

            # ---- collective bounce buffers --------------------------------
            ar_emb_in = dram.tile([128, 512], F32)
            ar_emb_out = dram.tile([128, 512], F32)
            a2a_q_in = dram.tile([B, 128], F32)
            a2a_q_out = dram.tile([B, 128], F32)
            ag_ctx_in = dram.tile([BL, H], F32)
            ag_ctx_out = dram.tile([B, H], F32)
            ag_nh_in = dram.tile([FSL, B], F32)
            ag_nh_out = dram.tile([H, B], F32)
            ag_hn_in = dram.tile([FSL, B], F32)
            ag_hn_out = dram.tile([H, B], F32)
            ag_st_in = dram.tile([B, 2], F32)
            ag_st_out = dram.tile([B * NC, 2], F32)

            # ---- phase 0: embedding gather + AllReduce --------------------
            idx_sb = pp.tile([B, 1], I32)
            nc.scalar.dma_start(idx_sb[:], idx_safe[:])
            emb_nat = pp.tile([B, E], F32)
            nc.gpsimd.indirect_dma_start(
                out=emb_nat[:],
                out_offset=None,
                in_=emb_sh[:],
                in_offset=bass.IndirectOffsetOnAxis(ap=idx_sb[:, :1], axis=0),
            )
            # transpose the local partial embedding BEFORE the AllReduce so
            # no PE work sits after the collective on the critical path
            embT_loc = pp.tile([128, 512], F32)
            for c in range(8):
                pt = ps.tile([128, 128], F32, tag="ps_t", name="pt")[:, :B]
                nc.tensor.transpose(pt[:], emb_nat[:, 128 * c:128 * (c + 1)],
                                    ident[:B, :B])
                nc.vector.tensor_copy(embT_loc[:, B * c:B * (c + 1)], pt[:])
            nc.scalar.dma_start(ar_emb_in[:], embT_loc[:])
            nc.gpsimd.collective_compute(
                "AllReduce", ALU.add, replica_groups=RG,
                ins=[ar_emb_in.opt()], outs=[ar_emb_out.opt()],
            )
            embT_all = pp.tile([128, 512], F32)
            nc.scalar.dma_start(embT_all[:], ar_emb_out[:])

            hT_all = pp.tile([128, 512], F32)
            nc.scalar.dma_start(hT_all[:], hT_in[:])

            # ---- phase 1: W_red (column-sharded) --------------------------
            pq = psg.tile([128, B], F32, tag="ps_g", name="pq")
            for k in list(range(8, 16)) + list(range(8)):
                wt = wp.tile([128, 128], F32, tag="wred")
                nc.sync.dma_start(wt[:], W_red_s[k])
                rhs = embT_all if k < 8 else hT_all
                kc = k % 8
                nc.tensor.matmul(pq[:], lhsT=wt[:], rhs=rhs[:, B * kc:B * (kc + 1)],
                                 start=(k == 8), stop=(k == 7))
            b_red_sb = pp.tile([128, 1], F32)
            nc.scalar.dma_start(b_red_sb[:], b_red_s[:])
            q_sl = pp.tile([128, B], F32)
            nc.scalar.activation(q_sl[:], pq[:], AF.Identity, bias=b_red_sb[:, 0:1])

            # ---- phase 2: AllToAll q (feature-shard -> batch-shard) -------
            ptq = ps.tile([128, 128], F32, tag="ps_t", name="pt")[:B, :]
            nc.tensor.transpose(ptq[:], q_sl[:], ident[:])
            q_nat_sl = pp.tile([B, 128], F32)
            nc.vector.tensor_copy(q_nat_sl[:], ptq[:])
            nc.scalar.dma_start(a2a_q_in[:], q_nat_sl[:])
            nc.gpsimd.collective_compute(
                "AllToAll", ALU.bypass, replica_groups=RG,
                ins=[a2a_q_in.opt()], outs=[a2a_q_out.opt()],
            )
            q_own_nat = pp.tile([BL, E], F32)
            for c in range(NC):
                nc.scalar.dma_start(q_own_nat[:, 128 * c:128 * (c + 1)],
                                  a2a_q_out[BL * c:BL * (c + 1), :])
            # qT_all: [128, 8*8], h-chunk c at cols 8c
            qT_all = pp.tile([128, 64], F32)
            for c in range(8):
                pt = ps.tile([128, 128], F32, tag="ps_t", name="pt")[:, :BL]
                nc.tensor.transpose(pt[:], q_own_nat[:, 128 * c:128 * (c + 1)],
                                    ident[:BL, :BL])
                nc.vector.tensor_copy(qT_all[:, BL * c:BL * (c + 1)], pt[:])

            # ---- phase 3: attention (batch-sharded) -----------------------
            # Work on "spread" layouts: batch b=4r+j lives on partition 32j
            # (PE column groups); other partitions hold garbage and are
            # masked to harmless values or never read.
            lens_sp_sb = pp.tile([128, 2], F32)
            nc.scalar.dma_start(lens_sp_sb[:], lens_sp[:])
            iota_i = pp.tile([128, S], I32)
            nc.gpsimd.iota(iota_i[:], pattern=[[1, S]], base=0, channel_multiplier=0)
            iota_f = pp.tile([128, S], F32)
            nc.vector.tensor_copy(iota_f[:], iota_i[:])

            attnT_sp = []
            for r in range(2):
                pen = pp.tile([128, S], F32, tag=f"pen{r}", name=f"pen{r}")
                nc.vector.tensor_scalar(pen[:], iota_f[:], lens_sp_sb[:, r:r + 1],
                                        1e30, ALU.is_ge, ALU.mult)
                pss = psb.tile([128, S], F32, tag="ps_big", name="psbt")
                for c in range(8):
                    for j in range(4):
                        b = 4 * r + j
                        et = ep_sc.tile([128, S], F32, tag="encT")
                        nc.sync.dma_start(et[:], encT_sc[b, c])
                        nc.tensor.matmul(
                            pss[32 * j:32 * j + 1, :],
                            lhsT=qT_all[:, 8 * c + b:8 * c + b + 1],
                            rhs=et[:],
                            start=(c == 0), stop=(c == 7),
                            tile_position=(0, 32 * j),
                        )
                # masked softmax on the spread rows (free-dim ops, rowwise)
                sc_m = pp.tile([128, S], F32, tag=f"sc_m{r}", name=f"sc_m{r}")
                nc.vector.tensor_tensor(out=sc_m[:], in0=pss[:], in1=pen[:],
                                        op=ALU.subtract)
                mx = pp.tile([128, 1], F32, tag=f"mx{r}", name=f"mx{r}")
                nc.vector.reduce_max(mx[:], sc_m[:], axis=mybir.AxisListType.X)
                nmx = pp.tile([128, 1], F32, tag=f"nmx{r}", name=f"nmx{r}")
                nc.vector.tensor_scalar_mul(nmx[:], mx[:], -1.0)
                ssum = pp.tile([128, 1], F32, tag=f"ssum{r}", name=f"ssum{r}")
                nc.scalar.activation(sc_m[:], sc_m[:], AF.Exp, bias=nmx[:, 0:1],
                                     accum_out=ssum[:])
                sinv = pp.tile([128, 1], F32, tag=f"sinv{r}", name=f"sinv{r}")
                nc.vector.reciprocal(sinv[:], ssum[:])
                nc.vector.tensor_scalar_mul(sc_m[:], sc_m[:], sinv[:, 0:1])
                # transpose: attnT_sp[r][:, 128*sc + 32*j] = attn weights
                at = pp.tile([128, 512], F32, tag=f"attnT{r}", name=f"attnT{r}")
                for sc in range(4):
                    pt = ps.tile([128, 128], F32, tag="ps_t", name="pt")
                    nc.tensor.transpose(pt[:], sc_m[:, 128 * sc:128 * (sc + 1)],
                                        ident[:])
                    nc.vector.tensor_copy(at[:, 128 * sc:128 * (sc + 1)], pt[:])
                attnT_sp.append(at)

            for r in range(2):
                psc0 = psb.tile([128, 512], F32, tag="ps_big", name="psc0")
                psc1 = psb.tile([128, 512], F32, tag="ps_big", name="psc1")
                psc = [psc0, psc1]
                for sc in range(4):
                    for j in range(4):
                        b = 4 * r + j
                        ct = ep_ct.tile([128, 1024], F32, tag="enc_ct")
                        nc.sync.dma_start(ct[:], enc_ct[b, sc])
                        for ht in range(2):
                            nc.tensor.matmul(
                                psc[ht][32 * j:32 * j + 1, :],
                                lhsT=attnT_sp[r][:, 128 * sc + 32 * j:
                                                 128 * sc + 32 * j + 1],
                                rhs=ct[:, 512 * ht:512 * (ht + 1)],
                                start=(sc == 0), stop=(sc == 3),
                                tile_position=(0, 32 * j),
                            )
                for j in range(4):
                    stg = wp.tile([1, H], F32, tag="ctx_stage", name="stg", bufs=2)
                    for ht in range(2):
                        nc.vector.tensor_copy(stg[:1, 512 * ht:512 * (ht + 1)],
                                              psc[ht][32 * j:32 * j + 1, :])
                    nc.scalar.dma_start(ag_ctx_in[4 * r + j:4 * r + j + 1, :],
                                      stg[:1, :])

            # ---- AllGather context -> full, transpose ---------------------
            nc.gpsimd.collective_compute(
                "AllGather", ALU.bypass, replica_groups=RG,
                ins=[ag_ctx_in.opt()], outs=[ag_ctx_out.opt()],
            )
            ctx_full = pp.tile([B, H], F32)
            nc.scalar.dma_start(ctx_full[:], ag_ctx_out[:])
            ctxT_all = pp.tile([128, 512], F32)
            for c in range(8):
                pt = ps.tile([128, 128], F32, tag="ps_t", name="pt")[:, :B]
                nc.tensor.transpose(pt[:], ctx_full[:, 128 * c:128 * (c + 1)],
                                    ident[:B, :B])
                nc.vector.tensor_copy(ctxT_all[:, B * c:B * (c + 1)], pt[:])

            # ---- phase 4: W_comb (column-sharded) -------------------------
            pnh = psg.tile([128, B], F32, tag="ps_g", name="pnh")
            for k in list(range(8, 16)) + list(range(8)):
                wt = wp.tile([128, 128], F32, tag="wcomb")
                nc.sync.dma_start(wt[:], W_comb_s[k])
                rhs = ctxT_all if k < 8 else hT_all
                kc = k % 8
                nc.tensor.matmul(pnh[:], lhsT=wt[:], rhs=rhs[:, B * kc:B * (kc + 1)],
                                 start=(k == 8), stop=(k == 7))
            b_comb_sb = pp.tile([128, 1], F32)
            nc.scalar.dma_start(b_comb_sb[:], b_comb_s[:])
            nhT_own = pp.tile([128, B], F32)
            nc.scalar.activation(nhT_own[:], pnh[:], AF.Identity,
                                 bias=b_comb_sb[:, 0:1])

            # ---- phase 5: AllGather new_hidden ----------------------------
            nc.scalar.dma_start(ag_nh_in[:], nhT_own[:])
            nc.gpsimd.collective_compute(
                "AllGather", ALU.bypass, replica_groups=RG,
                ins=[ag_nh_in.opt()], outs=[ag_nh_out.opt()],
            )
            nhT_all = pp.tile([128, 512], F32)
            for c in range(8):
                nc.scalar.dma_start(nhT_all[:, B * c:B * (c + 1)],
                                  ag_nh_out[128 * c:128 * (c + 1), :])

            # ---- phase 6: GRU (column-sharded) ----------------------------
            b_rz_sb = pp.tile([128, 2], F32)
            nc.scalar.dma_start(b_rz_sb[:], b_rz[:])
            b_ni_sb = pp.tile([128, 1], F32)
            nc.scalar.dma_start(b_ni_sb[:], b_ni[:])
            b_nh_sb = pp.tile([128, 1], F32)
            nc.scalar.dma_start(b_nh_sb[:], b_nh[:])

            pr = psg.tile([128, B], F32, tag="ps_g", name="pr")
            pz = psg.tile([128, B], F32, tag="ps_g", name="pz")
            pni = psg.tile([128, B], F32, tag="ps_g", name="pni")
            phn = psg.tile([128, B], F32, tag="ps_g", name="phn")
            for k in range(8):
                wt = wp.tile([128, 384], F32, tag="wih", bufs=3, name="wtih")
                nc.sync.dma_start(wt[:], W_ih_s[k])
                rhsk = embT_all[:, B * k:B * (k + 1)]
                nc.tensor.matmul(pr[:], lhsT=wt[:, 0:128], rhs=rhsk,
                                 start=(k == 0), stop=False)
                nc.tensor.matmul(pz[:], lhsT=wt[:, 128:256], rhs=rhsk,
                                 start=(k == 0), stop=False)
                nc.tensor.matmul(pni[:], lhsT=wt[:, 256:384], rhs=rhsk,
                                 start=(k == 0), stop=(k == 7))
            for k in range(8):
                wt2 = wp.tile([128, 384], F32, tag="whh", bufs=3, name="wthh")
                nc.sync.dma_start(wt2[:], W_hh_s[k])
                rhsk = nhT_all[:, B * k:B * (k + 1)]
                nc.tensor.matmul(pr[:], lhsT=wt2[:, 0:128], rhs=rhsk,
                                 start=False, stop=(k == 7))
                nc.tensor.matmul(pz[:], lhsT=wt2[:, 128:256], rhs=rhsk,
                                 start=False, stop=(k == 7))
                nc.tensor.matmul(phn[:], lhsT=wt2[:, 256:384], rhs=rhsk,
                                 start=(k == 0), stop=(k == 7))
            r_sb = pp.tile([128, B], F32)
            nc.scalar.activation(r_sb[:], pr[:], AF.Sigmoid, bias=b_rz_sb[:, 0:1])
            z_sb = pp.tile([128, B], F32)
            nc.scalar.activation(z_sb[:], pz[:], AF.Sigmoid, bias=b_rz_sb[:, 1:2])
            ni_sb = pp.tile([128, B], F32)
            nc.scalar.activation(ni_sb[:], pni[:], AF.Identity, bias=b_ni_sb[:, 0:1])
            hn_sb = pp.tile([128, B], F32)
            nc.scalar.activation(hn_sb[:], phn[:], AF.Identity, bias=b_nh_sb[:, 0:1])

            rhn = pp.tile([128, B], F32)
            nc.vector.tensor_tensor(out=rhn[:], in0=r_sb[:], in1=hn_sb[:], op=ALU.mult)
            npre = pp.tile([128, B], F32)
            nc.vector.tensor_tensor(out=npre[:], in0=ni_sb[:], in1=rhn[:], op=ALU.add)
            n_sb = pp.tile([128, B], F32)
            nc.scalar.activation(n_sb[:], npre[:], AF.Tanh)
            # h_new = n + z*(nh - n)
            dlt = pp.tile([128, B], F32)
            nc.vector.tensor_tensor(out=dlt[:], in0=nhT_own[:], in1=n_sb[:],
                                    op=ALU.subtract)
            zd = pp.tile([128, B], F32)
            nc.vector.tensor_tensor(out=zd[:], in0=z_sb[:], in1=dlt[:], op=ALU.mult)
            hnT_own = pp.tile([128, B], F32)
            nc.vector.tensor_tensor(out=hnT_own[:], in0=n_sb[:], in1=zd[:], op=ALU.add)

            # ---- phase 7: AllGather h_new ---------------------------------
            nc.scalar.dma_start(ag_hn_in[:], hnT_own[:])
            nc.gpsimd.collective_compute(
                "AllGather", ALU.bypass, replica_groups=RG,
                ins=[ag_hn_in.opt()], outs=[ag_hn_out.opt()],
            )
            hnT_all = pp.tile([128, 512], F32)
            hnT_bf = pp.tile([128, 512], BF16)
            for c in range(8):
                nc.scalar.dma_start(hnT_all[:, B * c:B * (c + 1)],
                                  ag_hn_out[128 * c:128 * (c + 1), :])
                nc.vector.tensor_copy(hnT_bf[:, B * c:B * (c + 1)],
                                      hnT_all[:, B * c:B * (c + 1)])

            # hidden_out: transpose back to [B, H] and write
            hout_sb = pp.tile([B, H], F32)
            for c in range(8):
                pt = ps.tile([128, 128], F32, tag="ps_t", name="pt")[:B, :]
                nc.tensor.transpose(pt[:], hnT_all[:, B * c:B * (c + 1)], ident[:])
                nc.vector.tensor_copy(hout_sb[:, 128 * c:128 * (c + 1)], pt[:])
            nc.scalar.dma_start(hidden_out[:], hout_sb[:])

            # ---- phase 8: logits (vocab-sharded) --------------------------
            logits = pp.tile([B, OSH], F32)
            esums = pp.tile([B, 16], F32)
            n_tiles = (OSH + 511) // 512
            for nt in range(n_tiles):
                n0 = 512 * nt
                nsz = min(512, OSH - n0)
                pl = psb.tile([128, 512], F32, tag="ps_big", name="psbt")[:B, :]
                for k in range(8):
                    wt = op_.tile([128, 512], BF16, tag="wout")
                    nc.sync.dma_start(wt[:, :nsz], W_out_s[k, :, n0:n0 + nsz])
                    nc.tensor.matmul(pl[:, :nsz],
                                     lhsT=hnT_bf[:, B * k:B * (k + 1)],
                                     rhs=wt[:, :nsz], start=(k == 0),
                                     stop=(k == 7))
                bt = wp.tile([B, 512], F32, tag="bout", bufs=2)
                nc.sync.dma_start(bt[:, :nsz], b_out_s[:, n0:n0 + nsz])
                nc.vector.tensor_tensor(
                    out=logits[:, n0:n0 + nsz], in0=pl[:, :nsz],
                    in1=bt[:, :nsz], op=ALU.add)
                # per-tile exp + row-sum (no max shift: |logits| is O(10))
                pt = wp.tile([B, 512], F32, tag="probs", bufs=2)
                nc.scalar.activation(pt[:, :nsz], logits[:, n0:n0 + nsz], AF.Exp,
                                     accum_out=esums[:, nt:nt + 1])

            # ---- phase 9: log_softmax with global sum ---------------------
            sl = pp.tile([B, 1], F32)
            nc.vector.reduce_sum(sl[:], esums[:, :n_tiles],
                                 axis=mybir.AxisListType.X)
            nc.scalar.dma_start(ag_st_in[:, 0:1], sl[:])
            nc.gpsimd.collective_compute(
                "AllGather", ALU.bypass, replica_groups=RG,
                ins=[ag_st_in.opt()], outs=[ag_st_out.opt()],
            )
            stats_all = pp.tile([B, NC], F32)
            for rr in range(NC):
                nc.scalar.dma_start(stats_all[:, rr:rr + 1],
                                  ag_st_out[B * rr:B * (rr + 1), 0:1])
            sg = pp.tile([B, 1], F32)
            nc.vector.reduce_sum(sg[:], stats_all[:], axis=mybir.AxisListType.X)
            lse = pp.tile([B, 1], F32)
            nc.scalar.activation(lse[:], sg[:], AF.Ln)
            nc.vector.tensor_scalar(logits[:], logits[:], lse[:, 0:1], None,
                                    ALU.subtract)
            nc.sync.dma_start(out_slice[:], logits[:])

    nc.compile()
    return nc


_NC_CACHE = None


def _get_bass():
    global _NC_CACHE
    if _NC_CACHE is None:
        _NC_CACHE = build_bass()
    return _NC_CACHE


def _prep_in_maps(inputs, hidden, encoder_outputs, encoder_lengths, emb,
                  W_red, b_red, W_comb, b_comb, W_ih, b_ih, W_hh, b_hh,
                  W_out, b_out):
    f32 = np.float32
    idx = np.asarray(inputs).reshape(B).astype(np.int64)
    hT_in = np.ascontiguousarray(
        np.asarray(hidden, f32)[0].T.reshape(8, 128, B).transpose(1, 0, 2)
        .reshape(128, 512))
    enc = np.asarray(encoder_outputs, f32)
    W_out = np.asarray(W_out, f32)
    W_out_pad = np.zeros((H, OPAD), f32)
    W_out_pad[:, :V] = W_out
    b_out_pad = np.full((OPAD,), NEG, f32)
    b_out_pad[:V] = np.asarray(b_out, f32)
    W_red = np.asarray(W_red, f32)
    W_comb = np.asarray(W_comb, f32)
    W_ih = np.asarray(W_ih, f32)
    W_hh = np.asarray(W_hh, f32)
    b_ih = np.asarray(b_ih, f32)
    b_hh = np.asarray(b_hh, f32)
    emb = np.asarray(emb, f32)
    lens_all = np.asarray(encoder_lengths).reshape(B).astype(f32)

    in_maps = []
    for i in range(NC):
        lo = OSH * i
        hi = min(V, OSH * (i + 1))
        emb_sh = np.zeros((OSH + 1, E), f32)
        emb_sh[:hi - lo] = emb[lo:hi]
        loc = idx - lo
        idx_safe = np.where((loc >= 0) & (loc < hi - lo), loc, OSH) \
            .astype(np.int32).reshape(B, 1)

        bsl = slice(BL * i, BL * (i + 1))
        lens_sp = np.zeros((128, 2), f32)
        for r in range(2):
            for j in range(4):
                lens_sp[32 * j, r] = lens_all[BL * i + 4 * r + j]
        e = enc[:, bsl, :]                                   # [512, 8, 1024]
        encT_sc = np.ascontiguousarray(e.transpose(1, 2, 0)).reshape(BL, 8, 128, S)
        enc_ct = np.ascontiguousarray(e.transpose(1, 0, 2)).reshape(BL, 4, 128, H)

        fsl = slice(FSL * i, FSL * (i + 1))
        W_red_s = np.ascontiguousarray(W_red[:, fsl]).reshape(16, 128, 128)
        W_comb_s = np.ascontiguousarray(W_comb[:, fsl]).reshape(16, 128, 128)
        o0 = FSL * i
        cols = np.r_[o0:o0 + FSL, H + o0:H + o0 + FSL, 2 * H + o0:2 * H + o0 + FSL]
        W_ih_s = np.ascontiguousarray(W_ih[:, cols]).reshape(8, 128, 384)
        W_hh_s = np.ascontiguousarray(W_hh[:, cols]).reshape(8, 128, 384)
        b_rz = np.stack([b_ih[o0:o0 + FSL] + b_hh[o0:o0 + FSL],
                         b_ih[H + o0:H + o0 + FSL] + b_hh[H + o0:H + o0 + FSL]],
                        axis=1).astype(f32)
        b_ni = b_ih[2 * H + o0:2 * H + o0 + FSL].reshape(FSL, 1).astype(f32)
        b_nh = b_hh[2 * H + o0:2 * H + o0 + FSL].reshape(FSL, 1).astype(f32)
        W_out_s = W_out_pad[:, lo:lo + OSH].astype(ml_dtypes.bfloat16) \
            .reshape(8, 128, OSH)
        b_out_s = np.broadcast_to(b_out_pad[lo:lo + OSH], (B, OSH)).copy()

        in_maps.append({
            "emb_sh": emb_sh,
            "idx_safe": idx_safe,
            "hT_in": hT_in,
            "encT_sc": encT_sc,
            "enc_ct": enc_ct,
            "lens_sp": lens_sp,
            "W_red_s": W_red_s,
            "b_red_s": np.asarray(b_red, f32)[fsl].reshape(FSL, 1),
            "W_comb_s": W_comb_s,
            "b_comb_s": np.asarray(b_comb, f32)[fsl].reshape(FSL, 1),
            "W_ih_s": W_ih_s,
            "W_hh_s": W_hh_s,
            "b_rz": b_rz,
            "b_ni": b_ni,
            "b_nh": b_nh,
            "W_out_s": W_out_s,
            "b_out_s": b_out_s,
        })
    return in_maps


last_exec_time_ns = None


def kernel(_profile=False, **inputs):
    global last_exec_time_ns
    nc = _get_bass()
    in_maps = _prep_in_maps(**inputs)
    kw = {}
    if _profile:
        kw["trace"] = True
    res = bass_utils.run_bass_kernel_spmd(nc, in_maps, core_ids=list(range(NC)), **kw)
    last_exec_time_ns = res.exec_time_ns
    out = np.concatenate([r["out_slice"] for r in res.results], axis=1)[:, :V]
    hidden_out = res.results[0]["hidden_out"].reshape(1, B, H)
    return np.ascontiguousarray(out), hidden_out
